# revision 8
# baseline (speedup 1.0000x reference)
"""ChebConvNet (K=1) Trainium2 kernel: 3x silu(x@W+b) -> logits -> log_softmax.

Sharding: data-parallel over nodes across 8 NeuronCores. x is padded from
200000 to 200704 rows (8 * 25088); each core processes its shard in a
transposed [feat, node] layout so the 128-wide feature dim sits on SBUF
partitions. Layers run as separate streaming phases (A0/A1/A2) over the
whole shard so consecutive macro tiles are independent and the scalar
(ACT) engine — the silu bottleneck — stays dense. The last matmul flips
back to the natural [node, class] layout (lhsT = h2 tile), and the
row-wise log_softmax runs there in a final phase (separate ACT table set
for exp/ln). The device writes output partition-major; the host
unscrambles.

edge_index is unused (ChebConv with K=1 ignores the graph).
"""

import numpy as np

import concourse.bacc as bacc
import concourse.mybir as mybir
import concourse.tile as tile
from concourse.tile import add_dep_helper
from concourse.bass_utils import run_bass_kernel_spmd

P = 128          # feature dim == SBUF partitions
C = 40           # classes
N_FULL = 200000
N_CORES = 8
NS = 25088       # nodes per core (padded: 8 * 25088 = 200704)
MT = 1536        # macro tile (nodes) for phases A0/A1
MACROS = [MT] * 16 + [512]               # 16*1536 + 512 = 25088
MACROS2 = [1024] * 24 + [512]            # phase A2 (psum shared with zp pool)
BCHUNKS = [32] * 6 + [4]              # phase-B chunks (node groups of 128)
NG = NS // P                          # 196 node groups per core

F32 = mybir.dt.float32
BF16 = mybir.dt.bfloat16
AF = mybir.ActivationFunctionType

_CACHE = {}


def _build():
    if "nc" in _CACHE:
        return _CACHE["nc"]
    nc = bacc.Bacc(None, target_bir_lowering=False)
    xT = nc.declare_dram_parameter("xT", [P, NS], BF16, isOutput=False)
    CB = 3 * 2 * P + 2 * C + 3 * 4 + 4 * 32 * C  # 5980 bytes/partition
    cd = nc.declare_dram_parameter("consts", [P, CB], mybir.dt.uint8, isOutput=False)
    # partition-major scratch layout; host unscrambles to [NS, C]
    out = nc.declare_dram_parameter("out", [P, NG * C], F32, isOutput=True)

    with tile.TileContext(nc) as tc:
        with (
            tc.tile_pool(name="const", bufs=1) as cpool,
            tc.tile_pool(name="xin", bufs=3) as xin,
            tc.tile_pool(name="h2s", bufs=2) as h2sp,
            tc.tile_pool(name="big", bufs=1) as bigp,
            tc.tile_pool(name="pb", bufs=3) as pbp,
            tc.tile_pool(name="ph", bufs=2, space="PSUM") as ph,
            tc.tile_pool(name="pz", bufs=2, space="PSUM") as pz,
        ):
            craw = cpool.tile([P, CB], mybir.dt.uint8, tag="craw")
            nc.sync.dma_start(craw[:], cd[:])
            off = 0
            Wt = []
            for i in range(3):
                Wt.append(craw[:, off : off + 2 * P].bitcast(BF16))
                off += 2 * P
            W3t = craw[:, off : off + 2 * C].bitcast(BF16)
            off += 2 * C
            bt = []
            for i in range(3):
                bt.append(craw[:, off : off + 4].bitcast(F32))
                off += 4
            b3t = craw[:, off : off + 4 * 32 * C].bitcast(F32)

            # whole-shard staging. eall reuses h0's slot (tag "bigA"),
            # which is free once phase A1 has consumed h0.
            h0 = bigp.tile([P, NS], BF16, tag="bigA", name="h0all")
            h1 = bigp.tile([P, NS], BF16, tag="bigB", name="h1all")
            zall = bigp.tile([P, NG * C], F32, tag="zall")
            sall = bigp.tile([P, NG], F32, tag="sall")
            lsall = bigp.tile([P, NG], F32, tag="lsall")

            # ---- Phase A0: h0 = silu(x @ W0 + b0) ----
            n0 = 0
            for mt in MACROS:
                xa = xin.tile([P, MT], BF16, tag="xa")
                nc.sync.dma_start(xa[:, :mt], xT[:, n0 : n0 + mt])
                hp = ph.tile([P, MT], F32, tag="hpsum", name="hp0")
                for j in range(0, mt, 512):
                    nc.tensor.matmul(
                        hp[:, j : j + 512], Wt[0], xa[:, j : j + 512],
                        start=True, stop=True,
                    )
                nc.scalar.activation(
                    h0[:, n0 : n0 + mt], hp[:, :mt], AF.Silu,
                    bias=bt[0], scale=1.0,
                )
                n0 += mt

            # ---- Phase A1: h1 = silu(h0 @ W1 + b1) ----
            n0 = 0
            for mt in MACROS:
                hp = ph.tile([P, MT], F32, tag="hpsum", name="hp1")
                for j in range(0, mt, 512):
                    nc.tensor.matmul(
                        hp[:, j : j + 512], Wt[1], h0[:, n0 + j : n0 + j + 512],
                        start=True, stop=True,
                    )
                nc.scalar.activation(
                    h1[:, n0 : n0 + mt], hp[:, :mt], AF.Silu,
                    bias=bt[1], scale=1.0,
                )
                n0 += mt

            # ---- Phase A2: h2 = silu(h1 @ W2 + b2); z = h2 @ W3 + b3 ----
            n0 = 0
            for mt in MACROS2:
                g0, gn = n0 // P, mt // P
                hp = ph.tile([P, 1024], F32, tag="hpsum", name="hp2")
                for j in range(0, mt, 512):
                    nc.tensor.matmul(
                        hp[:, j : j + 512], Wt[2], h1[:, n0 + j : n0 + j + 512],
                        start=True, stop=True,
                    )
                h2 = h2sp.tile([P, 1024], BF16, tag="h2")
                last_silu = nc.scalar.activation(
                    h2[:, :mt], hp[:, :mt], AF.Silu, bias=bt[2], scale=1.0
                )
                zp = pz.tile([P, 8 * C], F32, tag="zpsum")
                for g in range(gn):
                    nc.tensor.matmul(
                        zp[:, g * C : (g + 1) * C],
                        h2[:, g * P : (g + 1) * P],
                        W3t,
                        start=True, stop=True,
                    )
                nc.vector.tensor_add(
                    zall[:, g0 * C : (g0 + gn) * C], zp[:, : gn * C], b3t[:, : gn * C]
                )
                n0 += mt

            # ---- Phase B: log_softmax (ACT table set: natural_log_exp) ----
            # eall reuses h0's SBUF slot; all Exp before the single Ln.
            eall = bigp.tile([P, NG * C], BF16, tag="bigA", name="eall")
            g0 = 0
            for k, gn in enumerate(BCHUNKS):
                exp_i = nc.scalar.activation(
                    eall[:, g0 * C : (g0 + gn) * C],
                    zall[:, g0 * C : (g0 + gn) * C],
                    AF.Exp,
                )
                add_dep_helper(exp_i.ins, last_silu.ins, sync=True,
                               reason="exp after all silus (ACT table set)")
                nc.vector.reduce_sum(
                    sall[:, g0 : g0 + gn],
                    eall[:, g0 * C : (g0 + gn) * C].rearrange(
                        "p (g c) -> p g c", g=gn
                    ),
                    axis=mybir.AxisListType.X,
                )
                nc.scalar.activation(
                    lsall[:, g0 : g0 + gn], sall[:, g0 : g0 + gn], AF.Ln
                )
                o = pbp.tile([P, 32 * C], F32, tag="o")
                sub_engine = nc.gpsimd if k < 3 else nc.vector
                sub_engine.tensor_tensor(
                    o[:, : gn * C].rearrange("p (g c) -> p g c", g=gn),
                    zall[:, g0 * C : (g0 + gn) * C].rearrange(
                        "p (g c) -> p g c", g=gn
                    ),
                    lsall[:, g0 : g0 + gn].broadcast_to([P, gn, C]),
                    op=mybir.AluOpType.subtract,
                )
                nc.sync.dma_start(
                    out[:, g0 * C : (g0 + gn) * C], o[:, : gn * C]
                )
                g0 += gn
    nc.compile()
    _CACHE["nc"] = nc
    return nc


def _in_maps(x, W0, b0, W1, b1, W2, b2, W3, b3):
    import ml_dtypes

    x = np.asarray(x, dtype=np.float32)
    xpad = np.zeros((N_CORES * NS, P), dtype=ml_dtypes.bfloat16)
    xpad[:N_FULL] = x
    parts = [
        np.asarray(W0, np.float32).astype(ml_dtypes.bfloat16).view(np.uint8),
        np.asarray(W1, np.float32).astype(ml_dtypes.bfloat16).view(np.uint8),
        np.asarray(W2, np.float32).astype(ml_dtypes.bfloat16).view(np.uint8),
        np.asarray(W3, np.float32).astype(ml_dtypes.bfloat16).view(np.uint8),
        np.asarray(b0, np.float32).reshape(P, 1).view(np.uint8),
        np.asarray(b1, np.float32).reshape(P, 1).view(np.uint8),
        np.asarray(b2, np.float32).reshape(P, 1).view(np.uint8),
        np.ascontiguousarray(
            np.broadcast_to(np.tile(np.asarray(b3, np.float32), 32), (P, 32 * C))
        ).view(np.uint8),
    ]
    common = {"consts": np.ascontiguousarray(np.concatenate(parts, axis=1))}
    maps = []
    for c in range(N_CORES):
        shard = xpad[c * NS : (c + 1) * NS]
        maps.append({**common, "xT": np.ascontiguousarray(shard.T)})
    return maps


def _unscramble(res):
    # device out: [128, 196*40] with node = g*128 + p  ->  [25088, 40]
    outs = []
    for c in range(N_CORES):
        o = res.results[c]["out"].reshape(P, NG, C)
        outs.append(np.ascontiguousarray(o.transpose(1, 0, 2)).reshape(NS, C))
    return np.concatenate(outs, axis=0)[:N_FULL]


def kernel(**inputs):
    nc = _build()
    maps = _in_maps(
        inputs["x"],
        inputs["W0"], inputs["b0"],
        inputs["W1"], inputs["b1"],
        inputs["W2"], inputs["b2"],
        inputs["W3"], inputs["b3"],
    )
    res = run_bass_kernel_spmd(nc, maps, list(range(N_CORES)))
    return _unscramble(res)
